# revision 11
# baseline (speedup 1.0000x reference)
"""Trainium2 Bass kernel: dictionary cross-attention (cosine attention, softmax, weighted sum).

Computation (per batch b):
    q = x @ wq_w.T + wq_b            # (n, rc)
    k = td @ wk_w.T + wk_b           # (m, rc)
    v = td @ wv_w.T + wv_b           # (m, dim)
    attn = softmax( l2n(q) @ l2n(k).T * (1 + clip(scale,0,1)*ln(m)) )   # (n, m)
    out  = attn @ v                  # (n, dim)
Returns (out, attn).

Sharding: data-parallel over batch: 16 batches / 8 cores = 2 batches per core.
Weights replicated. Each core computes its own slice fully; host concatenates.
"""

import numpy as np
from contextlib import ExitStack

import concourse.bass as bass
from concourse import bacc
from concourse import mybir, tile
from concourse.bass_utils import run_bass_kernel_spmd

# ---------------------------------------------------------------- constants
B = 16
N = 16384  # tokens per batch image (128*128)
DIM = 192
RC = 10
M = 64
EPS2 = 1e-24  # clamp on sum-of-squares == (1e-12)^2 clamp on the norm
LOGM = float(np.float32(np.log(M)))

N_CORES = 8
B_PER_CORE = B // N_CORES  # 2
NTOK = B_PER_CORE * N      # 32768 tokens per core
TOK = 128                  # tokens per subtile (partition dim)
SUB = 4                    # subtiles per supertile
ST = TOK * SUB             # 512 tokens per supertile

F32 = mybir.dt.float32
AF = mybir.ActivationFunctionType
ALU = mybir.AluOpType

# dtype knobs for the hot matmul stages (accuracy/speed tradeoff)
CFG = {
    "x_dt": F32,       # dtype of x tiles + x transposes (F32 = exact transpose)
    "qmm_dt": mybir.dt.float32r,  # dtype of xT/wqT operands in the q matmul
    "logit_dt": F32,   # dtype of qT/knT operands in the logits matmul
    "out_dt": F32,     # dtype of expT/v operands in the out matmul
    "stages": 99,      # debug: truncate per-supertile pipeline after N stages
    "setup_stages": 99,  # debug: truncate per-batch setup after N stages
}


def _body(ctx, tc, io, n_batches, n_st):
    nc = tc.nc
    x_d, td_d, wqT_d, wkT_d, wvT_d, bqT_d, bk_d, bv_d, scale_d, ident_d, out_d, attn_d = io

    x_dt = CFG["x_dt"]
    qm_dt = CFG["qmm_dt"]
    lg_dt = CFG["logit_dt"]
    om_dt = CFG["out_dt"]

    const = ctx.enter_context(tc.tile_pool(name="const", bufs=1))

    ident_sb = const.tile([128, 128], F32)
    nc.sync.dma_start(ident_sb[:], ident_d[:])

    wqT0 = const.tile([128, RC], qm_dt)
    nc.sync.dma_start(wqT0[:], wqT_d[0:128, :])
    wqT1 = const.tile([64, RC], qm_dt)
    nc.sync.dma_start(wqT1[:], wqT_d[128:192, :])
    wkT0 = const.tile([128, RC], F32)
    nc.sync.dma_start(wkT0[:], wkT_d[0:128, :])
    wkT1 = const.tile([64, RC], F32)
    nc.sync.dma_start(wkT1[:], wkT_d[128:192, :])
    wvT0 = const.tile([128, DIM], F32)
    nc.sync.dma_start(wvT0[:], wvT_d[0:128, :])
    wvT1 = const.tile([64, DIM], F32)
    nc.sync.dma_start(wvT1[:], wvT_d[128:192, :])

    bqT_sb = const.tile([RC, 1], F32)
    nc.sync.dma_start(bqT_sb[:], bqT_d[:])
    bk_sb = const.tile([1, RC], F32)
    nc.sync.dma_start(bk_sb[:], bk_d[:])
    bv_sb = const.tile([1, DIM], F32)
    nc.sync.dma_start(bv_sb[:], bv_d[:])
    scale_sb = const.tile([M, 1], F32)
    nc.sync.dma_start(scale_sb[:], scale_d[:])

    ones_m = const.tile([1, M], F32)
    nc.vector.memset(ones_m[:], 1.0)

    # gamma = 1 + clip(scale, 0, 1) * ln(M)
    gam = const.tile([M, 1], F32)
    nc.vector.tensor_scalar(gam[:], scale_sb[:], 0.0, 1.0, ALU.max, ALU.min)
    nc.vector.tensor_scalar(gam[:], gam[:], LOGM, 1.0, ALU.mult, ALU.add)

    for b in range(n_batches):
        with ExitStack() as bctx:
            # ---------------- per-batch setup: k, v, scaled-normalized kT
            sp = bctx.enter_context(tc.tile_pool(name="sb_setup", bufs=1))
            sctx = bctx.enter_context(ExitStack())
            spp = sctx.enter_context(tc.tile_pool(name="ps_setup", bufs=1, space="PSUM"))

            if CFG["setup_stages"] < 1:
                continue
            td_sb = sp.tile([M, DIM], F32)
            nc.sync.dma_start(td_sb[:], td_d[b])

            tdT0_ps = spp.tile([128, M], F32)
            nc.tensor.transpose(tdT0_ps[:], td_sb[:, 0:128], ident_sb[0:M, 0:M])
            tdT1_ps = spp.tile([64, M], F32)
            nc.tensor.transpose(tdT1_ps[:], td_sb[:, 128:192], ident_sb[0:M, 0:M])
            tdT0 = sp.tile([128, M], F32)
            nc.vector.tensor_copy(tdT0[:], tdT0_ps[:])
            tdT1 = sp.tile([64, M], F32)
            nc.vector.tensor_copy(tdT1[:], tdT1_ps[:])

            if CFG["setup_stages"] < 2:
                continue
            k_ps = spp.tile([M, RC], F32)
            nc.tensor.matmul(k_ps[:], tdT0[:], wkT0[:], start=True, stop=False)
            nc.tensor.matmul(k_ps[:], tdT1[:], wkT1[:], start=False, stop=False)
            nc.tensor.matmul(k_ps[:], ones_m[:], bk_sb[:], start=False, stop=True)
            k_sb = sp.tile([M, RC], F32)
            nc.scalar.copy(k_sb[:], k_ps[:])

            if CFG["setup_stages"] < 3:
                continue
            ssqk = sp.tile([M, 1], F32)
            k_scr = sp.tile([M, RC], F32)
            nc.scalar.activation(k_scr[:], k_sb[:], AF.Square, accum_out=ssqk[:])
            nc.vector.tensor_scalar_max(ssqk[:], ssqk[:], EPS2)
            lnk = sp.tile([M, 1], F32)
            nc.scalar.activation(lnk[:], ssqk[:], AF.Ln)
            rnk = sp.tile([M, 1], F32)
            nc.scalar.activation(rnk[:], lnk[:], AF.Exp, scale=-0.5)

            if CFG["setup_stages"] < 4:
                continue
            kfac = sp.tile([M, 1], F32)
            nc.vector.tensor_mul(kfac[:], rnk[:], gam[:])
            knG = sp.tile([M, RC], F32)
            nc.vector.tensor_scalar_mul(knG[:], k_sb[:], kfac[:])
            knT_ps = spp.tile([RC, M], F32)
            nc.tensor.transpose(knT_ps[:], knG[:], ident_sb[0:M, 0:M])
            knT = sp.tile([RC, M], lg_dt)
            nc.vector.tensor_copy(knT[:], knT_ps[:])

            if CFG["setup_stages"] < 5:
                continue
            v_ps = spp.tile([M, DIM], F32)
            nc.tensor.matmul(v_ps[:], tdT0[:], wvT0[:], start=True, stop=False)
            nc.tensor.matmul(v_ps[:], tdT1[:], wvT1[:], start=False, stop=False)
            nc.tensor.matmul(v_ps[:], ones_m[:], bv_sb[:], start=False, stop=True)
            v_sb = sp.tile([M, DIM], om_dt)
            nc.scalar.copy(v_sb[:], v_ps[:])

            sctx.close()  # release the setup PSUM pool before the main loop

            # ---------------- main loop over supertiles of 512 tokens
            xp = bctx.enter_context(tc.tile_pool(name="sb_x", bufs=3))
            wp = bctx.enter_context(tc.tile_pool(name="sb_work", bufs=2))
            op = bctx.enter_context(tc.tile_pool(name="sb_out", bufs=3))
            pp_xT = bctx.enter_context(tc.tile_pool(name="ps_xT", bufs=2, space="PSUM"))
            pp_qT = bctx.enter_context(tc.tile_pool(name="ps_qT", bufs=1, space="PSUM"))
            pp_sm = bctx.enter_context(tc.tile_pool(name="ps_sm", bufs=2, space="PSUM"))
            pp_eT = bctx.enter_context(tc.tile_pool(name="ps_eT", bufs=1, space="PSUM"))
            pp_o = bctx.enter_context(tc.tile_pool(name="ps_o", bufs=2, space="PSUM"))

            for st in range(n_st):
                r0 = b * N + st * ST

                x_sb = xp.tile([128, SUB, DIM], x_dt, name="x_sb")
                x_view = x_d[r0:r0 + ST].rearrange("(g p) c -> p g c", p=128)
                if x_dt == F32:
                    nc.sync.dma_start(x_sb[:], x_view)
                else:
                    nc.gpsimd.dma_start(x_sb[:], x_view)  # SWDGE casts during DMA

                if CFG["stages"] < 1:
                    continue
                # ---- transpose x: xT0 [128c, 512tok], xT1 [64c, 512tok]
                xT0_ps = pp_xT.tile([128, SUB, 128], x_dt, name="xT0_ps", tag="xT")
                for g in range(SUB):
                    nc.tensor.matmul(xT0_ps[:, g, :], x_sb[:, g, 0:128], ident_sb[:],
                                     is_transpose=True, start=(g == 0), stop=(g == SUB - 1))
                xT0_sb = wp.tile([128, SUB, 128], qm_dt, name="xT0_sb")
                nc.vector.tensor_copy(xT0_sb[:], xT0_ps[:])

                xT1_ps = pp_xT.tile([64, SUB, 128], x_dt, name="xT1_ps", tag="xT")
                for g in range(SUB):
                    nc.tensor.matmul(xT1_ps[:, g, :], x_sb[:, g, 128:192], ident_sb[:],
                                     is_transpose=True, start=(g == 0), stop=(g == SUB - 1))
                xT1_sb = wp.tile([64, SUB, 128], qm_dt, name="xT1_sb")
                nc.scalar.copy(xT1_sb[:], xT1_ps[:])

                if CFG["stages"] < 2:
                    continue
                # ---- qT = wqT.T @ xT : [10, 512]
                qT_ps = pp_qT.tile([RC, SUB, 128], F32, name="qT_ps")
                nc.tensor.matmul(qT_ps[:], wqT0[:], xT0_sb[:], start=True, stop=False)
                nc.tensor.matmul(qT_ps[:], wqT1[:], xT1_sb[:], start=False, stop=True)
                qT_sb = wp.tile([RC, SUB, 128], lg_dt, name="qT_sb")
                nc.scalar.add(qT_sb[:], qT_ps[:], bqT_sb[:])  # + wq_b (per partition)

                if CFG["stages"] < 3:
                    continue
                # ---- per-token 1/|q|: transpose q back to token-major, square-reduce
                ssq_st = wp.tile([128, SUB], F32, name="ssq_st")
                for g in range(SUB):
                    qn_ps = pp_sm.tile([128, RC], lg_dt, name="qn_ps", tag="sm")
                    nc.tensor.matmul(qn_ps[:], qT_sb[:, g, :], ident_sb[0:RC, 0:RC],
                                     is_transpose=True, start=True, stop=True)
                    q_scr = wp.tile([128, RC], F32, name="q_scr")
                    nc.scalar.activation(q_scr[:], qn_ps[:], AF.Square,
                                         accum_out=ssq_st[:, g:g + 1])
                nc.vector.tensor_scalar_max(ssq_st[:], ssq_st[:], EPS2)
                ln_st = wp.tile([128, SUB], F32, name="ln_st")
                nc.scalar.activation(ln_st[:], ssq_st[:], AF.Ln)
                rnorm_st = wp.tile([128, SUB], F32, name="rnorm_st")
                nc.scalar.activation(rnorm_st[:], ln_st[:], AF.Exp, scale=-0.5)

                if CFG["stages"] < 4:
                    continue
                # ---- logits, exp (with fused 1/|q| scale), row sums
                exp_sb = wp.tile([128, SUB, M], F32, name="exp_sb")
                rsum_st = wp.tile([128, SUB], F32, name="rsum_st")
                for g in range(SUB):
                    log_ps = pp_sm.tile([128, M], F32, name="log_ps", tag="sm")
                    nc.tensor.matmul(log_ps[:], qT_sb[:, g, :], knT[:], start=True, stop=True)
                    nc.scalar.activation(exp_sb[:, g, :], log_ps[:], AF.Exp,
                                         scale=rnorm_st[:, g:g + 1],
                                         accum_out=rsum_st[:, g:g + 1])
                rinv_st = wp.tile([128, SUB], F32, name="rinv_st")
                nc.vector.reciprocal(rinv_st[:], rsum_st[:])

                if CFG["stages"] < 5:
                    continue
                # ---- attn output
                attn_sb = op.tile([128, SUB, M], F32, name="attn_sb")
                for g in range(SUB):
                    nc.vector.tensor_scalar_mul(attn_sb[:, g, :], exp_sb[:, g, :],
                                                rinv_st[:, g:g + 1])
                attn_view = attn_d[r0:r0 + ST].rearrange("(g p) m -> p g m", p=128)
                nc.sync.dma_start(attn_view, attn_sb[:])

                if CFG["stages"] < 6:
                    continue
                # ---- out = attn @ v  (normalize rows by rsum in the epilogue)
                eT_ps = pp_eT.tile([64, SUB, 128], F32, name="eT_ps")
                for g in range(SUB):
                    nc.tensor.matmul(eT_ps[:, g, :], exp_sb[:, g, :], ident_sb[:],
                                     is_transpose=True, start=(g == 0), stop=(g == SUB - 1))
                eT_sb = wp.tile([64, SUB, 128], om_dt, name="eT_sb")
                nc.vector.tensor_copy(eT_sb[:], eT_ps[:])

                out_sb = op.tile([128, SUB, DIM], F32, name="out_sb")
                for g in range(SUB):
                    o_ps = pp_o.tile([128, DIM], F32, name="o_ps")
                    nc.tensor.matmul(o_ps[:], eT_sb[:, g, :], v_sb[:], start=True, stop=True)
                    if g % 2 == 0:
                        nc.scalar.activation(out_sb[:, g, :], o_ps[:], AF.Copy,
                                             scale=rinv_st[:, g:g + 1])
                    else:
                        nc.vector.tensor_scalar_mul(out_sb[:, g, :], o_ps[:],
                                                    rinv_st[:, g:g + 1])
                out_view = out_d[r0:r0 + ST].rearrange("(g p) c -> p g c", p=128)
                nc.sync.dma_start(out_view, out_sb[:])


def build_nc(n_batches=B_PER_CORE, n_st=N // ST):
    nc = bacc.Bacc("TRN2", target_bir_lowering=False, debug=False)
    ntok = n_batches * N

    x_d = nc.dram_tensor("x", [ntok, DIM], F32, kind="ExternalInput").ap()
    td_d = nc.dram_tensor("td", [n_batches, M, DIM], F32, kind="ExternalInput").ap()
    wqT_d = nc.dram_tensor("wqT", [DIM, RC], CFG["qmm_dt"], kind="ExternalInput").ap()
    wkT_d = nc.dram_tensor("wkT", [DIM, RC], F32, kind="ExternalInput").ap()
    wvT_d = nc.dram_tensor("wvT", [DIM, DIM], F32, kind="ExternalInput").ap()
    bqT_d = nc.dram_tensor("bqT", [RC, 1], F32, kind="ExternalInput").ap()
    bk_d = nc.dram_tensor("bk", [1, RC], F32, kind="ExternalInput").ap()
    bv_d = nc.dram_tensor("bv", [1, DIM], F32, kind="ExternalInput").ap()
    scale_d = nc.dram_tensor("scale", [M, 1], F32, kind="ExternalInput").ap()
    ident_d = nc.dram_tensor("ident", [128, 128], F32, kind="ExternalInput").ap()

    out_d = nc.dram_tensor("out", [ntok, DIM], F32, kind="ExternalOutput").ap()
    attn_d = nc.dram_tensor("attn", [ntok, M], F32, kind="ExternalOutput").ap()

    io = (x_d, td_d, wqT_d, wkT_d, wvT_d, bqT_d, bk_d, bv_d, scale_d, ident_d,
          out_d, attn_d)
    with tile.TileContext(nc) as tc:
        with ExitStack() as ctx:
            _body(ctx, tc, io, n_batches, n_st)
    nc.compile()
    return nc


def make_in_maps(x, td, wq_w, wq_b, wk_w, wk_b, wv_w, wv_b, scale,
                 n_batches=B_PER_CORE, n_cores=N_CORES):
    f = np.float32
    shared = {
        "wqT": np.ascontiguousarray(np.asarray(wq_w, f).T),
        "wkT": np.ascontiguousarray(np.asarray(wk_w, f).T),
        "wvT": np.ascontiguousarray(np.asarray(wv_w, f).T),
        "bqT": np.ascontiguousarray(np.asarray(wq_b, f).reshape(RC, 1)),
        "bk": np.ascontiguousarray(np.asarray(wk_b, f).reshape(1, RC)),
        "bv": np.ascontiguousarray(np.asarray(wv_b, f).reshape(1, DIM)),
        "scale": np.ascontiguousarray(np.asarray(scale, f).reshape(M, 1)),
        "ident": np.eye(128, dtype=f),
    }
    x = np.asarray(x, f)
    td = np.asarray(td, f)
    in_maps = []
    for c in range(n_cores):
        bs = slice(c * n_batches, (c + 1) * n_batches)
        m = dict(shared)
        m["x"] = np.ascontiguousarray(x[bs].reshape(n_batches * N, DIM))
        m["td"] = np.ascontiguousarray(td[bs])
        in_maps.append(m)
    return in_maps


_NC_CACHE = {}


def _get_nc():
    key = (B_PER_CORE, N // ST, CFG["x_dt"], CFG["qmm_dt"], CFG["logit_dt"], CFG["out_dt"])
    if key not in _NC_CACHE:
        _NC_CACHE[key] = build_nc()
    return _NC_CACHE[key]


def run(inputs, trace=False, **kw):
    """Run the full problem on 8 cores; returns (out, attn, BassKernelResults)."""
    in_maps = make_in_maps(
        inputs["x"], inputs["td"], inputs["wq_w"], inputs["wq_b"],
        inputs["wk_w"], inputs["wk_b"], inputs["wv_w"], inputs["wv_b"],
        inputs["scale"])
    nc = _get_nc()
    res = run_bass_kernel_spmd(nc, in_maps, list(range(N_CORES)), trace=trace, **kw)
    out = np.concatenate(
        [res.results[c]["out"].reshape(B_PER_CORE, N, DIM) for c in range(N_CORES)], axis=0)
    attn = np.concatenate(
        [res.results[c]["attn"].reshape(B_PER_CORE, N, M) for c in range(N_CORES)], axis=0)
    return out, attn, res


def kernel(x, td, wq_w, wq_b, wk_w, wk_b, wv_w, wv_b, scale, h=128, w=128):
    inputs = {"x": x, "td": td, "wq_w": wq_w, "wq_b": wq_b, "wk_w": wk_w,
              "wk_b": wk_b, "wv_w": wv_w, "wv_b": wv_b, "scale": scale}
    out, attn, _ = run(inputs, trace=False)
    return out, attn


# revision 22
# speedup vs baseline: 5.5725x; 5.5725x over previous
"""Trainium2 Bass kernel: dictionary cross-attention (cosine attention, softmax, weighted sum).

Computation (per batch b):
    q = x @ wq_w.T + wq_b            # (n, rc)
    k = td @ wk_w.T + wk_b           # (m, rc)
    v = td @ wv_w.T + wv_b           # (m, dim)
    attn = softmax( l2n(q) @ l2n(k).T * (1 + clip(scale,0,1)*ln(m)) )   # (n, m)
    out  = attn @ v                  # (n, dim)
Returns (out, attn).

Sharding: data-parallel over batch: 16 batches / 8 cores = 2 batches per core.
Weights replicated. Each core computes its own slice fully; host concatenates.
"""

import numpy as np
from contextlib import ExitStack

import concourse.bass as bass
from concourse import bacc
from concourse import mybir, tile
from concourse.bass_utils import run_bass_kernel_spmd

# ---------------------------------------------------------------- constants
B = 16
N = 16384  # tokens per batch image (128*128)
DIM = 192
RC = 10
M = 64
EPS2 = 1e-24  # clamp on sum-of-squares == (1e-12)^2 clamp on the norm
LOGM = float(np.float32(np.log(M)))

N_CORES = 8
B_PER_CORE = B // N_CORES  # 2
NTOK = B_PER_CORE * N      # 32768 tokens per core
TOK = 128                  # tokens per subtile (partition dim)
SUB = 4                    # subtiles per supertile
ST = TOK * SUB             # 512 tokens per supertile

F32 = mybir.dt.float32
AF = mybir.ActivationFunctionType
ALU = mybir.AluOpType

# dtype knobs for the hot matmul stages (accuracy/speed tradeoff)
CFG = {
    "x_dt": F32,       # dtype of x tiles + x transposes (F32 = exact transpose)
    "qmm_dt": mybir.dt.float32r,  # dtype of xT/wqT operands in the q matmul
    "logit_dt": F32,   # dtype of qT/knT operands in the logits matmul
    "out_dt": F32,     # dtype of expT/v operands in the out matmul
    "vpad": 192,       # pad v free dim so fp32r out-matmul hits 1 cyc/row (N>=256)
    "stages": 99,      # debug: truncate per-supertile pipeline after N stages
    "setup_stages": 99,  # debug: truncate per-batch setup after N stages
}


def _body(ctx, tc, io, n_batches, n_st, repeat=1):
    nc = tc.nc
    x_d, td_d, wqT_d, wkT_d, wvT_d, bqT_d, bk_d, bv_d, scale_d, ident_d, out_d, attn_d = io

    x_dt = CFG["x_dt"]
    qm_dt = CFG["qmm_dt"]
    lg_dt = CFG["logit_dt"]
    om_dt = CFG["out_dt"]

    const = ctx.enter_context(tc.tile_pool(name="const", bufs=1))

    ident_sb = const.tile([128, 128], F32)
    nc.sync.dma_start(ident_sb[:], ident_d[:])

    wqT0 = const.tile([128, RC], qm_dt)
    nc.sync.dma_start(wqT0[:], wqT_d[0:128, :])
    wqT1 = const.tile([64, RC], qm_dt)
    nc.sync.dma_start(wqT1[:], wqT_d[128:192, :])
    wkT0 = const.tile([128, RC], F32)
    nc.sync.dma_start(wkT0[:], wkT_d[0:128, :])
    wkT1 = const.tile([64, RC], F32)
    nc.sync.dma_start(wkT1[:], wkT_d[128:192, :])
    wvT0 = const.tile([128, DIM], F32)
    nc.sync.dma_start(wvT0[:], wvT_d[0:128, :])
    wvT1 = const.tile([64, DIM], F32)
    nc.sync.dma_start(wvT1[:], wvT_d[128:192, :])

    bqT_sb = const.tile([RC, 1], F32)
    nc.sync.dma_start(bqT_sb[:], bqT_d[:])
    bk_sb = const.tile([1, RC], F32)
    nc.sync.dma_start(bk_sb[:], bk_d[:])
    bv_sb = const.tile([1, DIM], F32)
    nc.sync.dma_start(bv_sb[:], bv_d[:])
    scale_sb = const.tile([M, 1], F32)
    nc.sync.dma_start(scale_sb[:], scale_d[:])

    ones_m = const.tile([1, M], F32)
    nc.vector.memset(ones_m[:], 1.0)

    # gamma = 1 + clip(scale, 0, 1) * ln(M)
    gam = const.tile([M, 1], F32)
    nc.vector.tensor_scalar(gam[:], scale_sb[:], 0.0, 1.0, ALU.max, ALU.min)
    nc.vector.tensor_scalar(gam[:], gam[:], LOGM, 1.0, ALU.mult, ALU.add)

    for b in [bb for _ in range(repeat) for bb in range(n_batches)]:
        with ExitStack() as bctx:
            # ---------------- per-batch setup: k, v, scaled-normalized kT
            sp = bctx.enter_context(tc.tile_pool(name="sb_setup", bufs=1))
            sctx = bctx.enter_context(ExitStack())
            spp = sctx.enter_context(tc.tile_pool(name="ps_setup", bufs=1, space="PSUM"))

            if CFG["setup_stages"] < 1:
                continue
            td_sb = sp.tile([M, DIM], F32)
            nc.sync.dma_start(td_sb[:], td_d[b])

            tdT0_ps = spp.tile([128, M], F32)
            nc.tensor.transpose(tdT0_ps[:], td_sb[:, 0:128], ident_sb[0:M, 0:M])
            tdT1_ps = spp.tile([64, M], F32)
            nc.tensor.transpose(tdT1_ps[:], td_sb[:, 128:192], ident_sb[0:M, 0:M])
            tdT0 = sp.tile([128, M], F32)
            nc.vector.tensor_copy(tdT0[:], tdT0_ps[:])
            tdT1 = sp.tile([64, M], F32)
            nc.vector.tensor_copy(tdT1[:], tdT1_ps[:])

            if CFG["setup_stages"] < 2:
                continue
            k_ps = spp.tile([M, RC], F32)
            nc.tensor.matmul(k_ps[:], tdT0[:], wkT0[:], start=True, stop=False)
            nc.tensor.matmul(k_ps[:], tdT1[:], wkT1[:], start=False, stop=False)
            nc.tensor.matmul(k_ps[:], ones_m[:], bk_sb[:], start=False, stop=True)
            k_sb = sp.tile([M, RC], F32)
            nc.scalar.copy(k_sb[:], k_ps[:])

            if CFG["setup_stages"] < 3:
                continue
            ssqk = sp.tile([M, 1], F32)
            k_scr = sp.tile([M, RC], F32)
            nc.scalar.activation(k_scr[:], k_sb[:], AF.Square, accum_out=ssqk[:])
            nc.vector.tensor_scalar_max(ssqk[:], ssqk[:], EPS2)
            lnk = sp.tile([M, 1], F32)
            nc.scalar.activation(lnk[:], ssqk[:], AF.Ln)
            rnk = sp.tile([M, 1], F32)
            nc.scalar.activation(rnk[:], lnk[:], AF.Exp, scale=-0.5)

            if CFG["setup_stages"] < 4:
                continue
            kfac = sp.tile([M, 1], F32)
            nc.vector.tensor_mul(kfac[:], rnk[:], gam[:])
            knG = sp.tile([M, RC], F32)
            nc.vector.tensor_scalar_mul(knG[:], k_sb[:], kfac[:])
            knT_ps = spp.tile([RC, M], F32)
            nc.tensor.transpose(knT_ps[:], knG[:], ident_sb[0:M, 0:M])
            knT = sp.tile([RC, M], lg_dt)
            nc.vector.tensor_copy(knT[:], knT_ps[:])

            if CFG["setup_stages"] < 5:
                continue
            v_ps = spp.tile([M, DIM], F32)
            nc.tensor.matmul(v_ps[:], tdT0[:], wvT0[:], start=True, stop=False)
            nc.tensor.matmul(v_ps[:], tdT1[:], wvT1[:], start=False, stop=False)
            nc.tensor.matmul(v_ps[:], ones_m[:], bv_sb[:], start=False, stop=True)
            vpad = CFG["vpad"]
            v_sb = sp.tile([M, vpad], om_dt)
            if vpad > DIM:
                zpad = sp.tile([M, vpad - DIM], F32)
                nc.vector.memset(zpad[:], 0.0)
                nc.vector.tensor_copy(v_sb[:, DIM:vpad], zpad[:])
            nc.scalar.copy(v_sb[:, 0:DIM], v_ps[:])

            sctx.close()  # release the setup PSUM pool before the main loop

            # batch-persistent q state: qT for all tokens + 1/|q| per token
            qT_all = sp.tile([RC, n_st, SUB, 128], lg_dt)
            rnorm_all = sp.tile([128, n_st, SUB], F32)

            # ---------------- pass A: x -> qT, rnorm (whole batch)
            xp = bctx.enter_context(tc.tile_pool(name="sb_x", bufs=4))
            wp = bctx.enter_context(tc.tile_pool(name="sb_work", bufs=3))
            pa_ctx = bctx.enter_context(ExitStack())
            pp_xT = pa_ctx.enter_context(tc.tile_pool(name="ps_xT", bufs=3, space="PSUM"))
            pp_qT = pa_ctx.enter_context(tc.tile_pool(name="ps_qT", bufs=2, space="PSUM"))
            pp_qn = pa_ctx.enter_context(tc.tile_pool(name="ps_qn", bufs=2, space="PSUM"))

            for st in range(n_st):
                r0 = b * N + st * ST

                x_sb = xp.tile([128, SUB, DIM], x_dt, name="x_sb")
                x_view = x_d[r0:r0 + ST].rearrange("(g p) c -> p g c", p=128)
                if x_dt == F32:
                    nc.sync.dma_start(x_sb[:], x_view)
                else:
                    nc.gpsimd.dma_start(x_sb[:], x_view)  # SWDGE casts during DMA

                xT0_ps = pp_xT.tile([128, SUB, 128], x_dt, name="xT0_ps", tag="xT")
                for g in range(SUB):
                    nc.tensor.matmul(xT0_ps[:, g, :], x_sb[:, g, 0:128], ident_sb[:],
                                     is_transpose=True, start=(g == 0), stop=(g == SUB - 1))
                xT0_sb = wp.tile([128, SUB, 128], qm_dt, name="xT0_sb")
                nc.vector.tensor_copy(xT0_sb[:], xT0_ps[:])

                xT1_ps = pp_xT.tile([64, SUB, 128], x_dt, name="xT1_ps", tag="xT")
                for g in range(SUB):
                    nc.tensor.matmul(xT1_ps[:, g, :], x_sb[:, g, 128:192], ident_sb[:],
                                     is_transpose=True, start=(g == 0), stop=(g == SUB - 1))
                xT1_sb = wp.tile([64, SUB, 128], qm_dt, name="xT1_sb")
                nc.scalar.copy(xT1_sb[:], xT1_ps[:])

                qT_ps = pp_qT.tile([RC, SUB, 128], F32, name="qT_ps")
                nc.tensor.matmul(qT_ps[:], wqT0[:], xT0_sb[:], start=True, stop=False)
                nc.tensor.matmul(qT_ps[:], wqT1[:], xT1_sb[:], start=False, stop=True)
                nc.scalar.add(qT_all[:, st, :, :], qT_ps[:], bqT_sb[:])  # + wq_b

                qn_ps = pp_qn.tile([128, SUB, RC], lg_dt, name="qn_ps")
                for g in range(SUB):
                    nc.tensor.matmul(qn_ps[:, g, :], qT_all[:, st, g, :], ident_sb[0:RC, 0:RC],
                                     is_transpose=True, start=(g == 0), stop=(g == SUB - 1))
                q_scr = wp.tile([128, SUB, RC], F32, name="q_scr")
                nc.scalar.activation(q_scr[:], qn_ps[:], AF.Square)
                ssq_st = wp.tile([128, SUB], F32, name="ssq_st")
                nc.vector.reduce_sum(ssq_st[:], q_scr[:], axis=mybir.AxisListType.X)
                nc.vector.tensor_scalar_max(ssq_st[:], ssq_st[:], EPS2)
                ln_st = wp.tile([128, SUB], F32, name="ln_st")
                nc.scalar.activation(ln_st[:], ssq_st[:], AF.Ln)
                nc.scalar.activation(rnorm_all[:, st, :], ln_st[:], AF.Exp, scale=-0.5)

            pa_ctx.close()  # free pass-A PSUM pools

            # ---------------- pass B: logits -> softmax -> attn, out
            op = bctx.enter_context(tc.tile_pool(name="sb_out", bufs=4))
            pp_lg = bctx.enter_context(tc.tile_pool(name="ps_lg", bufs=3, space="PSUM"))
            pp_eT = bctx.enter_context(tc.tile_pool(name="ps_eT", bufs=2, space="PSUM"))
            pp_o = bctx.enter_context(tc.tile_pool(name="ps_o", bufs=3, space="PSUM"))

            for st in range(n_st):
                r0 = b * N + st * ST

                exp_sb = wp.tile([128, SUB, M], F32, name="exp_sb")
                rsum_st = wp.tile([128, SUB], F32, name="rsum_st")
                for g in range(SUB):
                    log_ps = pp_lg.tile([128, M], F32, name="log_ps")
                    nc.tensor.matmul(log_ps[:], qT_all[:, st, g, :], knT[:], start=True, stop=True)
                    nc.scalar.activation(exp_sb[:, g, :], log_ps[:], AF.Exp,
                                         scale=rnorm_all[:, st, g:g + 1],
                                         accum_out=rsum_st[:, g:g + 1])
                rinv_st = wp.tile([128, SUB], F32, name="rinv_st")
                nc.vector.reciprocal(rinv_st[:], rsum_st[:])

                attn_sb = op.tile([128, SUB, M], F32, name="attn_sb")
                for g in range(SUB):
                    nc.gpsimd.tensor_scalar_mul(attn_sb[:, g, :], exp_sb[:, g, :],
                                                rinv_st[:, g:g + 1])
                attn_view = attn_d[r0:r0 + ST].rearrange("(g p) m -> p g m", p=128)
                nc.scalar.dma_start(attn_view, attn_sb[:])

                eT_ps = pp_eT.tile([64, SUB, 128], F32, name="eT_ps")
                for g in range(SUB):
                    nc.tensor.matmul(eT_ps[:, g, :], exp_sb[:, g, :], ident_sb[:],
                                     is_transpose=True, start=(g == 0), stop=(g == SUB - 1))
                eT_sb = wp.tile([64, SUB, 128], om_dt, name="eT_sb")
                nc.vector.tensor_copy(eT_sb[:], eT_ps[:])

                out_sb = op.tile([128, SUB, DIM], F32, name="out_sb")
                for g in range(SUB):
                    o_ps = pp_o.tile([128, CFG["vpad"]], F32, name="o_ps")
                    nc.tensor.matmul(o_ps[:], eT_sb[:, g, :], v_sb[:], start=True, stop=True)
                    if g % 2 == 0:
                        nc.scalar.activation(out_sb[:, g, :], o_ps[:, 0:DIM], AF.Copy,
                                             scale=rinv_st[:, g:g + 1])
                    else:
                        nc.vector.tensor_scalar_mul(out_sb[:, g, :], o_ps[:, 0:DIM],
                                                    rinv_st[:, g:g + 1])
                out_view = out_d[r0:r0 + ST].rearrange("(g p) c -> p g c", p=128)
                nc.scalar.dma_start(out_view, out_sb[:])


def build_nc(n_batches=B_PER_CORE, n_st=N // ST, repeat=1):
    nc = bacc.Bacc("TRN2", target_bir_lowering=False, debug=False)
    ntok = n_batches * N

    x_d = nc.dram_tensor("x", [ntok, DIM], F32, kind="ExternalInput").ap()
    td_d = nc.dram_tensor("td", [n_batches, M, DIM], F32, kind="ExternalInput").ap()
    wqT_d = nc.dram_tensor("wqT", [DIM, RC], CFG["qmm_dt"], kind="ExternalInput").ap()
    wkT_d = nc.dram_tensor("wkT", [DIM, RC], F32, kind="ExternalInput").ap()
    wvT_d = nc.dram_tensor("wvT", [DIM, DIM], F32, kind="ExternalInput").ap()
    bqT_d = nc.dram_tensor("bqT", [RC, 1], F32, kind="ExternalInput").ap()
    bk_d = nc.dram_tensor("bk", [1, RC], F32, kind="ExternalInput").ap()
    bv_d = nc.dram_tensor("bv", [1, DIM], F32, kind="ExternalInput").ap()
    scale_d = nc.dram_tensor("scale", [M, 1], F32, kind="ExternalInput").ap()
    ident_d = nc.dram_tensor("ident", [128, 128], F32, kind="ExternalInput").ap()

    out_d = nc.dram_tensor("out", [ntok, DIM], F32, kind="ExternalOutput").ap()
    attn_d = nc.dram_tensor("attn", [ntok, M], F32, kind="ExternalOutput").ap()

    io = (x_d, td_d, wqT_d, wkT_d, wvT_d, bqT_d, bk_d, bv_d, scale_d, ident_d,
          out_d, attn_d)
    with tile.TileContext(nc) as tc:
        with ExitStack() as ctx:
            _body(ctx, tc, io, n_batches, n_st, repeat)
    nc.compile()
    return nc


def make_in_maps(x, td, wq_w, wq_b, wk_w, wk_b, wv_w, wv_b, scale,
                 n_batches=B_PER_CORE, n_cores=N_CORES):
    f = np.float32
    shared = {
        "wqT": np.ascontiguousarray(np.asarray(wq_w, f).T),
        "wkT": np.ascontiguousarray(np.asarray(wk_w, f).T),
        "wvT": np.ascontiguousarray(np.asarray(wv_w, f).T),
        "bqT": np.ascontiguousarray(np.asarray(wq_b, f).reshape(RC, 1)),
        "bk": np.ascontiguousarray(np.asarray(wk_b, f).reshape(1, RC)),
        "bv": np.ascontiguousarray(np.asarray(wv_b, f).reshape(1, DIM)),
        "scale": np.ascontiguousarray(np.asarray(scale, f).reshape(M, 1)),
        "ident": np.eye(128, dtype=f),
    }
    x = np.asarray(x, f)
    td = np.asarray(td, f)
    in_maps = []
    for c in range(n_cores):
        bs = slice(c * n_batches, (c + 1) * n_batches)
        m = dict(shared)
        m["x"] = np.ascontiguousarray(x[bs].reshape(n_batches * N, DIM))
        m["td"] = np.ascontiguousarray(td[bs])
        in_maps.append(m)
    return in_maps


_NC_CACHE = {}


def _get_nc():
    key = (B_PER_CORE, N // ST, CFG["x_dt"], CFG["qmm_dt"], CFG["logit_dt"], CFG["out_dt"])
    if key not in _NC_CACHE:
        _NC_CACHE[key] = build_nc()
    return _NC_CACHE[key]


def run(inputs, trace=False, **kw):
    """Run the full problem on 8 cores; returns (out, attn, BassKernelResults)."""
    in_maps = make_in_maps(
        inputs["x"], inputs["td"], inputs["wq_w"], inputs["wq_b"],
        inputs["wk_w"], inputs["wk_b"], inputs["wv_w"], inputs["wv_b"],
        inputs["scale"])
    nc = _get_nc()
    res = run_bass_kernel_spmd(nc, in_maps, list(range(N_CORES)), trace=trace, **kw)
    out = np.concatenate(
        [res.results[c]["out"].reshape(B_PER_CORE, N, DIM) for c in range(N_CORES)], axis=0)
    attn = np.concatenate(
        [res.results[c]["attn"].reshape(B_PER_CORE, N, M) for c in range(N_CORES)], axis=0)
    return out, attn, res


def kernel(x, td, wq_w, wq_b, wk_w, wk_b, wv_w, wv_b, scale, h=128, w=128):
    inputs = {"x": x, "td": td, "wq_w": wq_w, "wq_b": wq_b, "wk_w": wk_w,
              "wk_b": wk_b, "wv_w": wv_w, "wv_b": wv_b, "scale": scale}
    out, attn, _ = run(inputs, trace=False)
    return out, attn
